# revision 1
# baseline (speedup 1.0000x reference)
"""Mixture-of-Softmaxes kernel for 8 Trainium2 NeuronCores.

Strategy: tensor-parallel over the vocab dimension (V=100000 -> 12500/core).
Each core computes all B rows for its vocab shard: per-head logits via bf16
matmuls, exp via ScalarE (with row-sum side-accumulation), a tiny [128,1]
per-head AllReduce of the softmax denominators across cores, then the
pi-weighted mixture on VectorE. Output is gathered on the host by
concatenating the vocab shards (bf16 -> f32 cast on host).

Pipelining: exp values live in a ring of half-head tiles with a spare
slot, so ScalarE/TensorE stream into the next block while the current
block's mixture waits on its collectives. Mixture passes are gated
per-head so collective latency overlaps the remaining heads' compute.
projT is spilled to DRAM and per-block weight slices are prefetched a
block ahead to free SBUF for the ring.

Host-side prep: inputs are transposed (contraction dim -> SBUF partitions)
and cast to bf16 before DMA, so the kernel needs no on-chip transposes.
"""

import numpy as np
import ml_dtypes

import concourse.bass as bass
import concourse.mybir as mybir
import concourse.tile as tile
from concourse import bacc
from concourse.bass_utils import run_bass_kernel_spmd
from concourse.bass_interp import get_hw_module

B, H, D, V = 1024, 4, 256, 100000
N_CORES = 8
V_S = V // N_CORES          # 12500 vocab entries per core
KT = D // 128               # 2 contraction k-tiles
BBLK = 128                  # b rows per block (= SBUF partitions)
N_BBLK = B // BBLK          # 8 blocks
HALF = V_S // 2             # 6250: e-ring slot width
QRT = V_S // 4              # 3125: mixture/acc chunk width
E_SLOTS = 9                 # 8 per block + 1 slack (ScalarE runs ahead)

# psum chunking within a half: matmul N<=512 (one bank), ACT reads 4 banks
_HCHUNKS = [(0, 2048), (2048, 2048), (4096, 2048), (6144, 106)]

F32 = mybir.dt.float32
BF16 = mybir.dt.bfloat16

_RUN_KWARGS = {}  # test harness may set trace/tmpdir here
_CACHE = {}


def _build():
    nc = bacc.Bacc("TRN2", target_bir_lowering=False, debug=False,
                   num_devices=N_CORES)
    xT = nc.dram_tensor("xT", [D, B], BF16, kind="ExternalInput").ap()
    pmT = nc.dram_tensor("pmT", [D, H * D], BF16, kind="ExternalInput").ap()
    mmT = nc.dram_tensor("mmT", [D, H], BF16, kind="ExternalInput").ap()
    embT = nc.dram_tensor("embT", [D, V_S], BF16, kind="ExternalInput").ap()
    out = nc.dram_tensor("out", [B, V_S], BF16, kind="ExternalOutput").ap()

    with tile.TileContext(nc) as tc:
        _body(tc, xT, pmT, mmT, embT, out)
        tc._pool_ctx.close()

    nc.compile()
    nc.m = get_hw_module(nc.m)
    return nc


def _body(tc, xT, pmT, mmT, embT, out):
    nc = tc.nc
    Exp = mybir.ActivationFunctionType.Exp
    Tanh = mybir.ActivationFunctionType.Tanh
    add = mybir.AluOpType.add

    import contextlib
    ctx = contextlib.ExitStack()
    tc._pool_ctx = ctx
    singles = ctx.enter_context(tc.tile_pool(name="singles", bufs=1))
    work = ctx.enter_context(tc.tile_pool(name="work", bufs=3))
    mix = ctx.enter_context(tc.tile_pool(name="mix", bufs=2))
    lwp = ctx.enter_context(tc.tile_pool(name="lwp", bufs=2))
    ering = ctx.enter_context(tc.tile_pool(name="ering", bufs=E_SLOTS))
    psum = ctx.enter_context(tc.tile_pool(name="psum", bufs=2, space="PSUM"))
    dram = ctx.enter_context(tc.tile_pool(name="dram", bufs=4, space="DRAM"))

    # ---- resident SBUF inputs (xT/pmT borrow e-ring slots: prologue-only)
    sb_xT, sb_pmT, sb_mmT, sb_embT = [], [], [], []
    for k in range(KT):
        t = ering.tile([128, HALF], BF16, tag="e", name=f"xT{k}")
        nc.sync.dma_start(out=t[:, :B], in_=xT[k * 128:(k + 1) * 128, :])
        sb_xT.append(t[:, :B])
        t = ering.tile([128, HALF], BF16, tag="e", name=f"pmT{k}")
        nc.sync.dma_start(out=t[:, :H * D], in_=pmT[k * 128:(k + 1) * 128, :])
        sb_pmT.append(t[:, :H * D])
        t = work.tile([128, H], BF16, tag=f"mmT{k}", name=f"mmT{k}")
        nc.sync.dma_start(out=t, in_=mmT[k * 128:(k + 1) * 128, :])
        sb_mmT.append(t)
        t = singles.tile([128, V_S], BF16, tag=f"embT{k}", name=f"embT{k}")
        nc.sync.dma_start(out=t, in_=embT[k * 128:(k + 1) * 128, :])
        sb_embT.append(t)

    # ---- projT[h][kd] = tanh(proj_mat_h @ x.T), spilled to DRAM ----
    projT_dram = [[dram.tile([128, B], BF16, tag=f"pjd{h}_{kd}", bufs=1,
                             name=f"pjd{h}_{kd}")
                   for kd in range(KT)] for h in range(H)]
    for h in range(H):
        for kd in range(KT):
            for bs in range(B // 512):
                ps = psum.tile([128, 2048], F32, tag="ps", name="ps")
                for kc in range(KT):
                    nc.tensor.matmul(
                        ps[:, :512],
                        sb_pmT[kc][:, h * D + kd * 128: h * D + (kd + 1) * 128],
                        sb_xT[kc][:, bs * 512:(bs + 1) * 512],
                        start=(kc == 0), stop=(kc == KT - 1),
                    )
                stg = work.tile([128, 512], BF16, tag="stg", name="stg")
                nc.scalar.activation(out=stg, in_=ps[:, :512], func=Tanh)
                nc.sync.dma_start(
                    out=projT_dram[h][kd][:, bs * 512:(bs + 1) * 512],
                    in_=stg)

    # ---- pi[b, h] = softmax_h(x @ mix_mat.T) per b-block ----
    sb_pi = []
    for i in range(N_BBLK):
        ps = psum.tile([128, 2048], F32, tag="ps", name="ps")
        for kc in range(KT):
            nc.tensor.matmul(
                ps[:, :H],
                sb_xT[kc][:, i * 128:(i + 1) * 128],
                sb_mmT[kc],
                start=(kc == 0), stop=(kc == KT - 1),
            )
        m = work.tile([128, 1], F32, tag="pim", name="pim")
        nc.vector.tensor_reduce(out=m, in_=ps[:, :H],
                                axis=mybir.AxisListType.X,
                                op=mybir.AluOpType.max)
        negm = work.tile([128, 1], F32, tag="pinegm", name="pinegm")
        nc.vector.tensor_scalar_mul(negm, m, -1.0)
        e = work.tile([128, H], F32, tag="pie", name="pie")
        nc.scalar.activation(out=e, in_=ps[:, :H], func=Exp, bias=negm)
        s = work.tile([128, 1], F32, tag="pis", name="pis")
        nc.vector.tensor_reduce(out=s, in_=e, axis=mybir.AxisListType.X,
                                op=add)
        rs = work.tile([128, 1], F32, tag="pirs", name="pirs")
        nc.vector.reciprocal(rs, s)
        pi = singles.tile([128, H], F32, tag=f"pi{i}", name=f"pi{i}")
        nc.vector.tensor_scalar_mul(pi, e, rs)
        sb_pi.append(pi)

    # ---- main loop over b-blocks ----
    def load_weights(i):
        lw = {}
        for h in range(H):
            for kc in range(KT):
                t = lwp.tile([128, 128], BF16, tag=f"lw{h}_{kc}",
                             name=f"lw{h}_{kc}")
                nc.sync.dma_start(
                    out=t, in_=projT_dram[h][kc][:, i * 128:(i + 1) * 128])
                lw[(h, kc)] = t
        return lw

    lw_cur = load_weights(0)
    for i in range(N_BBLK):
        accs = [mix.tile([128, QRT], BF16, tag="acc", bufs=4, name=f"acc{q}")
                for q in range(4)]
        lw_next = None
        for h in range(H):
            sparts = work.tile([128, 8], F32, tag=f"sp{h}", name=f"sp{h}")
            ehalves = []
            for half in range(2):
                ehalf = ering.tile([128, HALF], BF16, tag="e",
                                   name=f"e{h}_{half}")
                ehalves.append(ehalf)
                for ci, (c0, cw) in enumerate(_HCHUNKS):
                    v0 = half * HALF + c0
                    ps = psum.tile([128, 2048], F32, tag="ps", name="ps")
                    for kc in range(KT):
                        for ns in range((cw + 511) // 512):
                            n0 = ns * 512
                            nw = min(512, cw - n0)
                            nc.tensor.matmul(
                                ps[:, n0:n0 + nw],
                                lw_cur[(h, kc)],
                                sb_embT[kc][:, v0 + n0:v0 + n0 + nw],
                                start=(kc == 0), stop=(kc == KT - 1),
                            )
                    if ci < 3:
                        nc.scalar.activation(
                            out=ehalf[:, c0:c0 + cw], in_=ps[:, :cw],
                            func=Exp,
                            accum_out=sparts[:, half * 4 + ci:
                                             half * 4 + ci + 1],
                        )
                    else:
                        # tail chunk: skip ScalarE's accum register read;
                        # the 106-wide row-sum goes to DVE (has slack)
                        nc.scalar.activation(
                            out=ehalf[:, c0:c0 + cw], in_=ps[:, :cw],
                            func=Exp)
                        nc.vector.tensor_reduce(
                            out=sparts[:, half * 4 + 3:half * 4 + 4],
                            in_=ehalf[:, c0:c0 + cw],
                            axis=mybir.AxisListType.X, op=add)
            if h == 0 and i + 1 < N_BBLK:
                # prefetch next block's weight slices during head 1
                lw_next = load_weights(i + 1)

            # head-h denominator -> AllReduce across vocab shards
            s_loc = work.tile([128, 1], F32, tag=f"sloc{h}", name=f"sloc{h}")
            nc.vector.tensor_reduce(
                out=s_loc, in_=sparts,
                axis=mybir.AxisListType.X, op=add)
            cc_in = dram.tile([128, 1], F32, tag=f"ccin{h}", name=f"ccin{h}")
            cc_out = dram.tile([128, 1], F32, tag=f"ccout{h}",
                               name=f"ccout{h}")
            nc.gpsimd.dma_start(out=cc_in[:], in_=s_loc)
            nc.gpsimd.collective_compute(
                "AllReduce", add,
                replica_groups=[list(range(N_CORES))],
                ins=[cc_in.opt()], outs=[cc_out.opt()],
            )
            s_glob = work.tile([128, 1], F32, tag=f"sglob{h}",
                               name=f"sglob{h}")
            # gpsimd queue, NOT sync: the sync FIFO carries the big
            # output DMAs whose sem-waits would head-of-line-block this
            # latency-critical read (measured: sync placement costs ~40us)
            nc.gpsimd.dma_start(out=s_glob, in_=cc_out[:])
            rS = work.tile([128, 1], F32, tag=f"rS{h}", name=f"rS{h}")
            nc.vector.reciprocal(rS, s_glob)
            w = work.tile([128, 1], F32, tag=f"w{h}", name=f"w{h}")
            nc.vector.tensor_mul(w, sb_pi[i][:, h:h + 1], rS)

            # mixture pass h (DVE): tensor_scalar at 4x bf16, adds at 2x
            for q in range(4):
                half, sub = divmod(q, 2)
                esl = ehalves[half][:, sub * QRT:(sub + 1) * QRT]
                if h == 0:
                    nc.vector.tensor_scalar_mul(accs[q], esl, w)
                else:
                    t1 = mix.tile([128, QRT], BF16, tag="t1", name="t1")
                    nc.vector.tensor_scalar_mul(t1, esl, w)
                    nc.vector.tensor_tensor(
                        out=accs[q], in0=accs[q], in1=t1, op=add)
                if h == H - 1:
                    nc.sync.dma_start(
                        out=out[i * 128:(i + 1) * 128,
                                q * QRT:(q + 1) * QRT],
                        in_=accs[q])
        if lw_next is not None:
            lw_cur = lw_next


def _get_nc():
    if "nc" not in _CACHE:
        _CACHE["nc"] = _build()
    return _CACHE["nc"]


def kernel(x, proj_mat, mix_mat, emb):
    nc = _get_nc()
    bf = ml_dtypes.bfloat16
    xT = np.ascontiguousarray(x.astype(bf).T)
    pmT = np.ascontiguousarray(proj_mat.astype(bf).T)
    mmT = np.ascontiguousarray(mix_mat.astype(bf).T)
    emb_bf = emb.astype(bf)
    in_maps = []
    for c in range(N_CORES):
        embT = np.ascontiguousarray(emb_bf[c * V_S:(c + 1) * V_S].T)
        in_maps.append({"xT": xT, "pmT": pmT, "mmT": mmT, "embT": embT})
    res = run_bass_kernel_spmd(nc, in_maps, list(range(N_CORES)),
                               **_RUN_KWARGS)
    _CACHE["last_result"] = res
    return np.concatenate(
        [res.results[c]["out"].astype(np.float32) for c in range(N_CORES)],
        axis=1)

